# revision 20
# baseline (speedup 1.0000x reference)
"""Multi-head causal attention with RoPE on 8 Trainium2 NeuronCores.

Sharding: tensor-parallel over heads. Each of the 8 cores owns 2 of the 16
heads (a 256-row slice of w_q/w_k/w_v and the matching 256-column slice of
w_o). x is replicated. Each core computes its partial output projection
outT_c = w_o_slice.T @ ctx_slice in transposed [d, s] layout; the host sums
the 8 partials and transposes back.

On-device layout strategy (per core):
  - x arrives transposed: xT [2048, 4096] so projections contract d on
    partitions. q/k are produced in [head_dim, s] layout (RoPE applied via
    partition-half swizzle on DVE); v in natural [s, head_dim] layout.
  - attention runs in "scoresT" geometry: scoresT[k, q] = kT_tile.T @ qT,
    exp on ScalarE over k-tile PAIRS (scale 1/sqrt(128) fused, no max
    subtraction - scaled scores are bounded ~|6.5| so fp32 exp is safe),
    PV accumulates ctxT[m, q] with v tiles stationary, softmax denominator
    via an all-ones [128,128] stationary matmul (gives the denominator
    pre-broadcast across partitions), then reciprocal_approx_fast and one
    multiply normalize the context.
  - matmul operands are bf16 (full PE rate); all accumulation is fp32 PSUM.
Causality: per 512-wide q block only the valid k tiles run; scores/exp/PV/
denominator on diagonal tiles run only over the valid column suffix, and the
single partial 128-col triangle is masked post-exp with a [128,128] mask.
"""

import math

import numpy as np

S = 4096
D = 2048
DT = D // 128           # 16 d-tiles
MLOC = 256              # head dims per core (2 heads x 128)
HLOC = 2                # heads per core
CH = 512                # s-chunk == attention q-block
NCORES = 8
SCALE = 1.0 / math.sqrt(128.0)


def _build(s_len):
    import concourse.mybir as mybir
    import concourse.tile as tile
    from concourse import bacc

    f32 = mybir.dt.float32
    bf16 = mybir.dt.bfloat16
    AF = mybir.ActivationFunctionType

    n_qb = s_len // CH
    n_st = s_len // 128

    # partition id is unused (inputs are host-sharded per core); disabling it
    # removes the per-engine TENSOR_LOADs from the kernel preamble
    nc = bacc.Bacc(enable_partition_id=False)

    xTt = nc.dram_tensor("xTt", [128, s_len // CH, DT, CH], bf16, kind="ExternalInput")
    w_qTt = nc.dram_tensor("w_qTt", [128, DT, MLOC], bf16, kind="ExternalInput")
    w_kTt = nc.dram_tensor("w_kTt", [128, DT, MLOC], bf16, kind="ExternalInput")
    w_vTt = nc.dram_tensor("w_vTt", [128, DT, MLOC], bf16, kind="ExternalInput")
    w_oTt = nc.dram_tensor("w_oTt", [128, 2, D], bf16, kind="ExternalInput")
    cosT = nc.dram_tensor("cosT", [128, s_len], f32, kind="ExternalInput")
    sinTs = nc.dram_tensor("sinTs", [128, s_len], f32, kind="ExternalInput")
    # local causal triangle for diagonal k-tiles: tri[k, q'] = 1 iff k <= q'.
    # With suffix-sliced exp, only the first 128 columns past each diagonal
    # tile's q-offset ever need masking, and that triangle is the same for
    # every diagonal tile.
    maskp = nc.dram_tensor("maskp", [128, 128], bf16, kind="ExternalInput")
    outT = nc.dram_tensor("outT", [D, s_len], f32, kind="ExternalOutput")

    from contextlib import ExitStack

    with tile.TileContext(nc) as tc:
        with ExitStack() as ctx:
            consts = ctx.enter_context(tc.tile_pool(name="consts", bufs=1))
            wpool = ctx.enter_context(tc.tile_pool(name="wpool", bufs=1))
            kvpool = ctx.enter_context(tc.tile_pool(name="kvpool", bufs=1))
            xpool = ctx.enter_context(tc.tile_pool(name="xpool", bufs=2))
            qpool = ctx.enter_context(tc.tile_pool(name="qpool", bufs=2))
            ropepool = ctx.enter_context(tc.tile_pool(name="ropepool", bufs=2))
            tmppool = ctx.enter_context(tc.tile_pool(name="tmppool", bufs=3))
            epool = ctx.enter_context(tc.tile_pool(name="epool", bufs=6))
            spool = ctx.enter_context(tc.tile_pool(name="spool", bufs=4))
            ctxnpool = ctx.enter_context(tc.tile_pool(name="ctxnpool", bufs=2))
            rfpool = ctx.enter_context(tc.tile_pool(name="rfpool", bufs=2))
            obuf = ctx.enter_context(tc.tile_pool(name="obuf", bufs=3))
            pp1 = ctx.enter_context(tc.tile_pool(name="pp1", bufs=2, space="PSUM"))
            psc = ctx.enter_context(tc.tile_pool(name="psc", bufs=2, space="PSUM"))
            pam = ctx.enter_context(tc.tile_pool(name="pam", bufs=2, space="PSUM"))

            # ---- weights needed immediately ----
            # fine-grained first transfers: the first q-projection matmul only
            # needs w_q[:, 0:2, :] and x[:, 0, 0:2, :], so land those first and
            # stream the rest behind them to cut the startup PE bubble.
            # Startup transfers split across the two HWDGE issue engines (Sync
            # and Scalar) so descriptor generation and the two HW queues run
            # in parallel: Sync streams w_q + x chunk 0 (first matmul deps),
            # Scalar streams cos/sin (needed by rope at ~14us), then w_k, w_v.
            w_q_sb = wpool.tile([128, DT, MLOC], bf16)
            nc.sync.dma_start(out=w_q_sb[:, 0:2], in_=w_qTt[:, 0:2, :])
            xbig0 = xpool.tile([128, DT, CH], bf16, tag="xt", name="xt_pre0")
            nc.sync.dma_start(out=xbig0[:, 0:2], in_=xTt[:, 0, 0:2])
            cos0 = ropepool.tile([128, CH], f32, tag="cos", name="cos_pre0")
            nc.scalar.dma_start(out=cos0, in_=cosT[:, 0:CH])
            sin0 = ropepool.tile([128, CH], f32, tag="sin", name="sin_pre0")
            nc.scalar.dma_start(out=sin0, in_=sinTs[:, 0:CH])
            nc.sync.dma_start(out=w_q_sb[:, 2:8], in_=w_qTt[:, 2:8, :])
            nc.sync.dma_start(out=xbig0[:, 2:8], in_=xTt[:, 0, 2:8])
            w_k_sb = wpool.tile([128, DT, MLOC], bf16)
            nc.scalar.dma_start(out=w_k_sb, in_=w_kTt[:, :, :])
            nc.sync.dma_start(out=w_q_sb[:, 8:DT], in_=w_qTt[:, 8:DT, :])
            nc.sync.dma_start(out=xbig0[:, 8:DT], in_=xTt[:, 0, 8:DT])
            w_v_sb = wpool.tile([128, DT, MLOC], bf16)
            nc.scalar.dma_start(out=w_v_sb, in_=w_vTt[:, :, :])
            ones_sb = consts.tile([128, 128], bf16)
            nc.vector.memset(ones_sb, 1.0)

            # HAM pre-warm: the PE clock gate opens only after ~3.4us of
            # sustained matmul activity. Burn that window on dummy matmuls
            # during the initial DMA wait so the real projections start at
            # full clock. Output goes to a scratch PSUM tile, never read.
            warm_sb = consts.tile([128, CH], bf16)
            nc.vector.memset(warm_sb, 0.0)
            warm_ps = pam.tile([128, CH], f32, tag="am", name="warm_ps")
            for w in range(9):
                nc.tensor.matmul(
                    warm_ps,
                    lhsT=ones_sb,
                    rhs=warm_sb,
                    start=True,
                    stop=True,
                    skip_group_check=True,
                )

            # ---- persistent per-core tensors ----
            kT_sb = []
            for h in range(HLOC):
                kt = kvpool.tile([128, s_len], bf16, name=f"kT{h}")
                kT_sb.append(kt)
            v_sb = kvpool.tile([128, n_st * MLOC], bf16)

            w_o_sb = None
            mask_sb = None

            def emit_projections(i, cos_t, sin_t, xbig):
                """QKV projections + rope for s-chunk i. Returns per-head q tiles."""
                s0 = i * CH
                q_t = []
                for h in range(HLOC):
                    for wname, w_sb in (("q", w_q_sb), ("k", w_k_sb)):
                        acc = pp1.tile(
                            [128, CH], f32, tag="pp1", name=f"acc_{wname}{h}_{i}"
                        )
                        for d in range(DT):
                            nc.tensor.matmul(
                                acc,
                                lhsT=w_sb[:, d, h * 128 : (h + 1) * 128],
                                rhs=xbig[:, d, :],
                                start=(d == 0),
                                stop=(d == DT - 1),
                            )
                        if wname == "q":
                            dst = qpool.tile(
                                [128, CH], bf16, tag=f"q{h}", name=f"q{h}_{i}"
                            )
                            q_t.append(dst)
                        else:
                            dst = kT_sb[h][:, s0 : s0 + CH]
                        # rope: dst = acc*cos + rot(acc)*sin_signed
                        t1 = tmppool.tile(
                            [128, CH], f32, tag="t1", name=f"t1_{wname}{h}_{i}"
                        )
                        nc.vector.tensor_mul(t1, acc, cos_t)
                        nc.vector.tensor_mul(dst[0:64], acc[64:128], sin_t[0:64])
                        nc.vector.tensor_mul(dst[64:128], acc[0:64], sin_t[64:128])
                        nc.vector.tensor_add(dst, dst, t1)

                for st in range(CH // 128):
                    vacc = pp1.tile([128, MLOC], f32, tag="pp1", name=f"vacc{st}_{i}")
                    for d in range(DT):
                        nc.tensor.matmul(
                            vacc,
                            lhsT=xbig[:, d, st * 128 : (st + 1) * 128],
                            rhs=w_v_sb[:, d, :],
                            start=(d == 0),
                            stop=(d == DT - 1),
                        )
                    gst = i * (CH // 128) + st
                    nc.vector.tensor_copy(v_sb[:, gst * MLOC : (gst + 1) * MLOC], vacc)
                return q_t

            def emit_attention(i, q_t):
                """Attention + output projection for q-block i."""
                s0 = i * CH
                nk = (i + 1) * (CH // 128)
                npair = nk // 2
                ctxn = []
                for h in range(HLOC):
                    ctx_ps = pam.tile([128, CH], f32, tag="am", name=f"ctx{h}_{i}")
                    den_ps = pam.tile([128, CH], f32, tag="am", name=f"den{h}_{i}")
                    for jp in range(npair):
                        p0 = 2 * jp - (CH // 128) * i  # diagonal pattern of half 0
                        qlo0 = max(p0, 0) * 128
                        sc = psc.tile(
                            [128, 2, CH], f32, tag="sc", name=f"sc{h}_{i}_{jp}"
                        )
                        for half in range(2):
                            j = 2 * jp + half
                            p = j - (CH // 128) * i
                            qlo = max(p, 0) * 128
                            nc.tensor.matmul(
                                sc[:, half, qlo:CH],
                                lhsT=kT_sb[h][:, j * 128 : (j + 1) * 128],
                                rhs=q_t[h][:, qlo:CH],
                                start=True,
                                stop=True,
                            )
                        # exp per pair on ScalarE. The first pair of a block is
                        # split per half to cut the scores->exp->PV pipeline
                        # fill latency; the rest run as one flat ACT (per-ACT
                        # overhead beats the column savings). On diagonal pairs
                        # the flat ACT eats stale PSUM in the pre-suffix
                        # columns, which the masks/memset below neutralize
                        # before anything reads them.
                        e = epool.tile([128, 2, CH], bf16, tag="e", name=f"e{h}_{i}_{jp}")
                        nc.scalar.activation(
                            e.rearrange("p a b -> p (a b)"),
                            sc.rearrange("p a b -> p (a b)"),
                            AF.Exp,
                            scale=SCALE,
                        )
                        if p0 >= 0:
                            # half 0: triangle at its diagonal tile
                            nc.vector.tensor_mul(
                                e[:, 0, qlo0 : qlo0 + 128],
                                e[:, 0, qlo0 : qlo0 + 128],
                                mask_sb,
                            )
                            # half 1: zero the 128 cols before its diagonal tile
                            # (so the pair-sum below stays causal), then triangle
                            nc.vector.memset(e[:, 1, qlo0 : qlo0 + 128], 0.0)
                            nc.vector.tensor_mul(
                                e[:, 1, qlo0 + 128 : qlo0 + 256],
                                e[:, 1, qlo0 + 128 : qlo0 + 256],
                                mask_sb,
                            )
                        # pair-sum for the denominator: one den matmul per pair
                        # (halves the ones-matmul count on the PE; the sum runs
                        # on the DVE, whose latency the scheduler hides better
                        # than GpSimd's)
                        s_p = spool.tile(
                            [128, CH], bf16, tag="sp", name=f"sp{h}_{i}_{jp}"
                        )
                        nc.vector.tensor_add(
                            s_p[:, qlo0:CH], e[:, 0, qlo0:CH], e[:, 1, qlo0:CH]
                        )
                        nc.tensor.matmul(
                            den_ps[:, qlo0:CH],
                            lhsT=ones_sb,
                            rhs=s_p[:, qlo0:CH],
                            start=(jp == 0),
                            stop=(jp == npair - 1),
                            skip_group_check=True,
                        )
                        for half in range(2):
                            j = 2 * jp + half
                            p = j - (CH // 128) * i
                            qlo = max(p, 0) * 128  # valid column suffix start
                            nc.tensor.matmul(
                                ctx_ps[:, qlo:CH],
                                lhsT=v_sb[
                                    :, j * MLOC + h * 128 : j * MLOC + (h + 1) * 128
                                ],
                                rhs=e[:, half, qlo:CH],
                                start=(j == 0),
                                stop=(j == nk - 1),
                                skip_group_check=True,
                            )
                    rf = rfpool.tile([128, CH], f32, tag="rf", name=f"rf{h}_{i}")
                    nc.vector.reciprocal_approx_fast(rf, den_ps)
                    cn = ctxnpool.tile([128, CH], bf16, tag=f"cn{h}", name=f"cn{h}_{i}")
                    nc.vector.tensor_mul(cn, ctx_ps, rf)
                    ctxn.append(cn)

                # ---- output projection for q-block i ----
                for t in range(DT):
                    o_ps = pam.tile([128, CH], f32, tag="am", name=f"o{t}_{i}")
                    for ot in range(2):
                        nc.tensor.matmul(
                            o_ps,
                            lhsT=w_o_sb[:, ot, t * 128 : (t + 1) * 128],
                            rhs=ctxn[ot],
                            start=(ot == 0),
                            stop=(ot == 1),
                        )
                    o_sb = obuf.tile([128, CH], f32, tag="osb", name=f"osb{t}_{i}")
                    # balance PSUM->SBUF drains across DVE and ScalarE: the
                    # copies otherwise congest DVE right when the next block's
                    # rope/normalize chain needs it
                    if t % 2 == 1:
                        nc.scalar.copy(o_sb, o_ps)
                    else:
                        nc.vector.tensor_copy(o_sb, o_ps)
                    nc.sync.dma_start(
                        out=outT[t * 128 : (t + 1) * 128, s0 : s0 + CH],
                        in_=o_sb,
                    )

            # ---- main loop ----
            # The Tile scheduler is readiness-driven per engine: later-emitted
            # projection matmuls naturally fill exp-wait bubbles inside the
            # current block's attention, so plain program order works best.
            for i in range(n_qb):
                if i == 0:
                    cos_t, sin_t, xbig = cos0, sin0, xbig0
                else:
                    s0 = i * CH
                    cos_t = ropepool.tile([128, CH], f32, tag="cos", name=f"cos{i}")
                    nc.sync.dma_start(out=cos_t, in_=cosT[:, s0 : s0 + CH])
                    sin_t = ropepool.tile([128, CH], f32, tag="sin", name=f"sin{i}")
                    nc.sync.dma_start(out=sin_t, in_=sinTs[:, s0 : s0 + CH])
                    xbig = xpool.tile([128, DT, CH], bf16, tag="xt", name=f"xt{i}")
                    nc.sync.dma_start(out=xbig, in_=xTt[:, i])
                q_t = emit_projections(i, cos_t, sin_t, xbig)
                if i == 0:
                    # lower-priority loads, after the first chunk's stream
                    w_o_sb = wpool.tile([128, 2, D], bf16)
                    nc.sync.dma_start(out=w_o_sb, in_=w_oTt[:, :, :])
                    mask_sb = consts.tile([128, 128], bf16)
                    nc.sync.dma_start(out=mask_sb, in_=maskp[:, :])
                emit_attention(i, q_t)

    nc.finalize()
    return nc


def _host_inputs(x, w_q, w_k, w_v, w_o, s_len):
    """Host-side sharding / layout prep. Returns per-core input maps."""
    import ml_dtypes

    bf = ml_dtypes.bfloat16
    x2 = np.ascontiguousarray(x.reshape(s_len, D).astype(np.float32))
    xT = np.ascontiguousarray(x2.T.astype(bf))

    half = 64
    inv_freq = 1.0 / (10000.0 ** (np.arange(half, dtype=np.float32) / half))
    pos = np.arange(s_len, dtype=np.float32)
    ang = pos[:, None] * inv_freq[None, :]
    ang = np.concatenate([ang, ang], axis=1)  # [s, 128]
    cosT = np.ascontiguousarray(np.cos(ang).T.astype(np.float32))
    sinTs = np.ascontiguousarray(np.sin(ang).T.astype(np.float32))
    sinTs[:half] *= -1.0

    kk = np.arange(128)[:, None]
    qq = np.arange(128)[None, :]
    maskp = (kk <= qq).astype(bf)  # [128, 128] local causal triangle

    # tiled layouts: [128, ...] partition-major so device DMAs are long
    # contiguous runs (descriptor-count-bound otherwise)
    xTt = np.ascontiguousarray(
        xT.reshape(16, 128, s_len // 512, 512).transpose(1, 2, 0, 3)
    )  # [128, n_ch, 16, 512]

    def wtile(wslice_T):  # [2048, 256] -> [128, 16, 256]
        return np.ascontiguousarray(wslice_T.reshape(16, 128, MLOC).transpose(1, 0, 2))

    in_maps = []
    for c in range(NCORES):
        rows = slice(MLOC * c, MLOC * (c + 1))
        w_oc = w_o[:, rows].T.astype(bf)  # [256, 2048]
        in_maps.append(
            {
                "xTt": xTt,
                "w_qTt": wtile(w_q[rows].T.astype(bf)),
                "w_kTt": wtile(w_k[rows].T.astype(bf)),
                "w_vTt": wtile(w_v[rows].T.astype(bf)),
                "w_oTt": np.ascontiguousarray(
                    w_oc.reshape(2, 128, D).transpose(1, 0, 2)
                ),
                "cosT": cosT,
                "sinTs": sinTs,
                "maskp": maskp,
            }
        )
    return in_maps


_NC_CACHE = {}


def kernel(x, w_q, w_k, w_v, w_o):
    from concourse.bass_utils import run_bass_kernel_spmd

    s_len = x.shape[1]
    if s_len not in _NC_CACHE:
        _NC_CACHE[s_len] = _build(s_len)
    nc = _NC_CACHE[s_len]

    in_maps = _host_inputs(
        np.asarray(x), np.asarray(w_q), np.asarray(w_k), np.asarray(w_v),
        np.asarray(w_o), s_len,
    )
    res = run_bass_kernel_spmd(nc, in_maps, core_ids=list(range(NCORES)))
    acc = np.zeros((D, s_len), dtype=np.float32)
    for r in res.results:
        acc += r["outT"]
    return np.ascontiguousarray(acc.T)[None].astype(np.float32)



# revision 21
# speedup vs baseline: 1.0048x; 1.0048x over previous
"""Multi-head causal attention with RoPE on 8 Trainium2 NeuronCores.

Sharding: tensor-parallel over heads. Each of the 8 cores owns 2 of the 16
heads (a 256-row slice of w_q/w_k/w_v and the matching 256-column slice of
w_o). x is replicated. Each core computes its partial output projection
outT_c = w_o_slice.T @ ctx_slice in transposed [d, s] layout; the host sums
the 8 partials and transposes back.

On-device layout strategy (per core):
  - x arrives transposed: xT [2048, 4096] so projections contract d on
    partitions. q/k are produced in [head_dim, s] layout (RoPE applied via
    partition-half swizzle on DVE); v in natural [s, head_dim] layout.
  - attention runs in "scoresT" geometry: scoresT[k, q] = kT_tile.T @ qT,
    exp on ScalarE over k-tile PAIRS (scale 1/sqrt(128) fused, no max
    subtraction - scaled scores are bounded ~|6.5| so fp32 exp is safe),
    PV accumulates ctxT[m, q] with v tiles stationary, softmax denominator
    via an all-ones [128,128] stationary matmul (gives the denominator
    pre-broadcast across partitions), then reciprocal_approx_fast and one
    multiply normalize the context.
  - matmul operands are bf16 (full PE rate); all accumulation is fp32 PSUM.
Causality: per 512-wide q block only the valid k tiles run; scores/exp/PV/
denominator on diagonal tiles run only over the valid column suffix, and the
single partial 128-col triangle is masked post-exp with a [128,128] mask.
"""

import math

import numpy as np

S = 4096
D = 2048
DT = D // 128           # 16 d-tiles
MLOC = 256              # head dims per core (2 heads x 128)
HLOC = 2                # heads per core
CH = 512                # s-chunk == attention q-block
NCORES = 8
SCALE = 1.0 / math.sqrt(128.0)


def _build(s_len):
    import concourse.mybir as mybir
    import concourse.tile as tile
    from concourse import bacc

    f32 = mybir.dt.float32
    bf16 = mybir.dt.bfloat16
    AF = mybir.ActivationFunctionType

    n_qb = s_len // CH
    n_st = s_len // 128

    # partition id is unused (inputs are host-sharded per core); disabling it
    # removes the per-engine TENSOR_LOADs from the kernel preamble
    nc = bacc.Bacc(enable_partition_id=False)

    xTt = nc.dram_tensor("xTt", [128, s_len // CH, DT, CH], bf16, kind="ExternalInput")
    w_qTt = nc.dram_tensor("w_qTt", [128, DT, MLOC], bf16, kind="ExternalInput")
    w_kTt = nc.dram_tensor("w_kTt", [128, DT, MLOC], bf16, kind="ExternalInput")
    w_vTt = nc.dram_tensor("w_vTt", [128, DT, MLOC], bf16, kind="ExternalInput")
    w_oTt = nc.dram_tensor("w_oTt", [128, 2, D], bf16, kind="ExternalInput")
    cosT = nc.dram_tensor("cosT", [128, s_len], f32, kind="ExternalInput")
    sinTs = nc.dram_tensor("sinTs", [128, s_len], f32, kind="ExternalInput")
    # local causal triangle for diagonal k-tiles: tri[k, q'] = 1 iff k <= q'.
    # With suffix-sliced exp, only the first 128 columns past each diagonal
    # tile's q-offset ever need masking, and that triangle is the same for
    # every diagonal tile.
    maskp = nc.dram_tensor("maskp", [128, 128], bf16, kind="ExternalInput")
    outT = nc.dram_tensor("outT", [D, s_len], f32, kind="ExternalOutput")

    from contextlib import ExitStack

    with tile.TileContext(nc) as tc:
        with ExitStack() as ctx:
            consts = ctx.enter_context(tc.tile_pool(name="consts", bufs=1))
            wpool = ctx.enter_context(tc.tile_pool(name="wpool", bufs=1))
            kvpool = ctx.enter_context(tc.tile_pool(name="kvpool", bufs=1))
            xpool = ctx.enter_context(tc.tile_pool(name="xpool", bufs=2))
            qpool = ctx.enter_context(tc.tile_pool(name="qpool", bufs=2))
            ropepool = ctx.enter_context(tc.tile_pool(name="ropepool", bufs=2))
            tmppool = ctx.enter_context(tc.tile_pool(name="tmppool", bufs=3))
            epool = ctx.enter_context(tc.tile_pool(name="epool", bufs=6))
            spool = ctx.enter_context(tc.tile_pool(name="spool", bufs=4))
            ctxnpool = ctx.enter_context(tc.tile_pool(name="ctxnpool", bufs=2))
            rfpool = ctx.enter_context(tc.tile_pool(name="rfpool", bufs=2))
            obuf = ctx.enter_context(tc.tile_pool(name="obuf", bufs=3))
            pp1 = ctx.enter_context(tc.tile_pool(name="pp1", bufs=2, space="PSUM"))
            psc = ctx.enter_context(tc.tile_pool(name="psc", bufs=2, space="PSUM"))
            pam = ctx.enter_context(tc.tile_pool(name="pam", bufs=2, space="PSUM"))

            # ---- weights needed immediately ----
            # fine-grained first transfers: the first q-projection matmul only
            # needs w_q[:, 0:2, :] and x[:, 0, 0:2, :], so land those first and
            # stream the rest behind them to cut the startup PE bubble.
            # Startup transfers split across the two HWDGE issue engines (Sync
            # and Scalar) so descriptor generation and the two HW queues run
            # in parallel: Sync streams w_q + x chunk 0 (first matmul deps),
            # Scalar streams cos/sin (needed by rope at ~14us), then w_k, w_v.
            w_q_sb = wpool.tile([128, DT, MLOC], bf16)
            nc.sync.dma_start(out=w_q_sb[:, 0:2], in_=w_qTt[:, 0:2, :])
            xbig0 = xpool.tile([128, DT, CH], bf16, tag="xt", name="xt_pre0")
            nc.sync.dma_start(out=xbig0[:, 0:2], in_=xTt[:, 0, 0:2])
            cos0 = ropepool.tile([128, CH], f32, tag="cos", name="cos_pre0")
            nc.scalar.dma_start(out=cos0, in_=cosT[:, 0:CH])
            sin0 = ropepool.tile([128, CH], f32, tag="sin", name="sin_pre0")
            nc.scalar.dma_start(out=sin0, in_=sinTs[:, 0:CH])
            nc.sync.dma_start(out=w_q_sb[:, 2:8], in_=w_qTt[:, 2:8, :])
            nc.sync.dma_start(out=xbig0[:, 2:8], in_=xTt[:, 0, 2:8])
            w_k_sb = wpool.tile([128, DT, MLOC], bf16)
            nc.scalar.dma_start(out=w_k_sb, in_=w_kTt[:, :, :])
            nc.sync.dma_start(out=w_q_sb[:, 8:DT], in_=w_qTt[:, 8:DT, :])
            nc.sync.dma_start(out=xbig0[:, 8:DT], in_=xTt[:, 0, 8:DT])
            w_v_sb = wpool.tile([128, DT, MLOC], bf16)
            nc.scalar.dma_start(out=w_v_sb, in_=w_vTt[:, :, :])
            ones_sb = consts.tile([128, 128], bf16)
            nc.vector.memset(ones_sb, 1.0)

            # HAM pre-warm: the PE clock gate opens only after ~3.4us of
            # sustained matmul activity. Burn that window on dummy matmuls
            # during the initial DMA wait so the real projections start at
            # full clock. Output goes to a scratch PSUM tile, never read.
            warm_sb = consts.tile([128, CH], bf16)
            nc.vector.memset(warm_sb, 0.0)
            warm_ps = pam.tile([128, CH], f32, tag="am", name="warm_ps")
            for w in range(9):
                nc.tensor.matmul(
                    warm_ps,
                    lhsT=ones_sb,
                    rhs=warm_sb,
                    start=True,
                    stop=True,
                    skip_group_check=True,
                )

            # ---- persistent per-core tensors ----
            kT_sb = []
            for h in range(HLOC):
                kt = kvpool.tile([128, s_len], bf16, name=f"kT{h}")
                kT_sb.append(kt)
            v_sb = kvpool.tile([128, n_st * MLOC], bf16)

            w_o_sb = None
            mask_sb = None

            def emit_projections(i, cos_t, sin_t, xbig):
                """QKV projections + rope for s-chunk i. Returns per-head q tiles."""
                s0 = i * CH
                q_t = []
                for h in range(HLOC):
                    for wname, w_sb in (("q", w_q_sb), ("k", w_k_sb)):
                        acc = pp1.tile(
                            [128, CH], f32, tag="pp1", name=f"acc_{wname}{h}_{i}"
                        )
                        for d in range(DT):
                            nc.tensor.matmul(
                                acc,
                                lhsT=w_sb[:, d, h * 128 : (h + 1) * 128],
                                rhs=xbig[:, d, :],
                                start=(d == 0),
                                stop=(d == DT - 1),
                            )
                        if wname == "q":
                            dst = qpool.tile(
                                [128, CH], bf16, tag=f"q{h}", name=f"q{h}_{i}"
                            )
                            q_t.append(dst)
                        else:
                            dst = kT_sb[h][:, s0 : s0 + CH]
                        # rope: dst = acc*cos + rot(acc)*sin_signed
                        t1 = tmppool.tile(
                            [128, CH], f32, tag="t1", name=f"t1_{wname}{h}_{i}"
                        )
                        nc.vector.tensor_mul(t1, acc, cos_t)
                        nc.vector.tensor_mul(dst[0:64], acc[64:128], sin_t[0:64])
                        nc.vector.tensor_mul(dst[64:128], acc[0:64], sin_t[64:128])
                        nc.vector.tensor_add(dst, dst, t1)

                for st in range(CH // 128):
                    vacc = pp1.tile([128, MLOC], f32, tag="pp1", name=f"vacc{st}_{i}")
                    for d in range(DT):
                        nc.tensor.matmul(
                            vacc,
                            lhsT=xbig[:, d, st * 128 : (st + 1) * 128],
                            rhs=w_v_sb[:, d, :],
                            start=(d == 0),
                            stop=(d == DT - 1),
                        )
                    gst = i * (CH // 128) + st
                    nc.vector.tensor_copy(v_sb[:, gst * MLOC : (gst + 1) * MLOC], vacc)
                return q_t

            def emit_attention(i, q_t):
                """Attention + output projection for q-block i."""
                s0 = i * CH
                nk = (i + 1) * (CH // 128)
                npair = nk // 2
                ctxn = []
                for h in range(HLOC):
                    ctx_ps = pam.tile([128, CH], f32, tag="am", name=f"ctx{h}_{i}")
                    den_ps = pam.tile([128, CH], f32, tag="am", name=f"den{h}_{i}")
                    for jp in range(npair):
                        p0 = 2 * jp - (CH // 128) * i  # diagonal pattern of half 0
                        qlo0 = max(p0, 0) * 128
                        sc = psc.tile(
                            [128, 2, CH], f32, tag="sc", name=f"sc{h}_{i}_{jp}"
                        )
                        for half in range(2):
                            j = 2 * jp + half
                            p = j - (CH // 128) * i
                            qlo = max(p, 0) * 128
                            nc.tensor.matmul(
                                sc[:, half, qlo:CH],
                                lhsT=kT_sb[h][:, j * 128 : (j + 1) * 128],
                                rhs=q_t[h][:, qlo:CH],
                                start=True,
                                stop=True,
                            )
                        # exp per pair on ScalarE. The first pair of a block is
                        # split per half to cut the scores->exp->PV pipeline
                        # fill latency; the rest run as one flat ACT (per-ACT
                        # overhead beats the column savings). On diagonal pairs
                        # the flat ACT eats stale PSUM in the pre-suffix
                        # columns, which the masks/memset below neutralize
                        # before anything reads them.
                        e = epool.tile([128, 2, CH], bf16, tag="e", name=f"e{h}_{i}_{jp}")
                        nc.scalar.activation(
                            e.rearrange("p a b -> p (a b)"),
                            sc.rearrange("p a b -> p (a b)"),
                            AF.Exp,
                            scale=SCALE,
                        )
                        if p0 >= 0:
                            # half 0: triangle at its diagonal tile
                            nc.vector.tensor_mul(
                                e[:, 0, qlo0 : qlo0 + 128],
                                e[:, 0, qlo0 : qlo0 + 128],
                                mask_sb,
                            )
                            # half 1: zero the 128 cols before its diagonal tile
                            # (so the pair-sum below stays causal), then triangle
                            nc.vector.memset(e[:, 1, qlo0 : qlo0 + 128], 0.0)
                            nc.vector.tensor_mul(
                                e[:, 1, qlo0 + 128 : qlo0 + 256],
                                e[:, 1, qlo0 + 128 : qlo0 + 256],
                                mask_sb,
                            )
                        # pair-sum for the denominator: one den matmul per pair
                        # (halves the ones-matmul count on the PE; the sum runs
                        # on the DVE, whose latency the scheduler hides better
                        # than GpSimd's)
                        s_p = spool.tile(
                            [128, CH], bf16, tag="sp", name=f"sp{h}_{i}_{jp}"
                        )
                        nc.vector.tensor_add(
                            s_p[:, qlo0:CH], e[:, 0, qlo0:CH], e[:, 1, qlo0:CH]
                        )
                        nc.tensor.matmul(
                            den_ps[:, qlo0:CH],
                            lhsT=ones_sb,
                            rhs=s_p[:, qlo0:CH],
                            start=(jp == 0),
                            stop=(jp == npair - 1),
                            skip_group_check=True,
                        )
                        for half in range(2):
                            j = 2 * jp + half
                            p = j - (CH // 128) * i
                            qlo = max(p, 0) * 128  # valid column suffix start
                            nc.tensor.matmul(
                                ctx_ps[:, qlo:CH],
                                lhsT=v_sb[
                                    :, j * MLOC + h * 128 : j * MLOC + (h + 1) * 128
                                ],
                                rhs=e[:, half, qlo:CH],
                                start=(j == 0),
                                stop=(j == nk - 1),
                                skip_group_check=True,
                            )
                    rf = rfpool.tile([128, CH], f32, tag="rf", name=f"rf{h}_{i}")
                    nc.vector.reciprocal_approx_fast(rf, den_ps)
                    cn = ctxnpool.tile([128, CH], bf16, tag=f"cn{h}", name=f"cn{h}_{i}")
                    nc.vector.tensor_mul(cn, ctx_ps, rf)
                    ctxn.append(cn)

                # ---- output projection for q-block i ----
                for t in range(DT):
                    o_ps = pam.tile([128, CH], f32, tag="am", name=f"o{t}_{i}")
                    for ot in range(2):
                        nc.tensor.matmul(
                            o_ps,
                            lhsT=w_o_sb[:, ot, t * 128 : (t + 1) * 128],
                            rhs=ctxn[ot],
                            start=(ot == 0),
                            stop=(ot == 1),
                        )
                    o_sb = obuf.tile([128, CH], f32, tag="osb", name=f"osb{t}_{i}")
                    # PSUM->SBUF drains stay on DVE except in the last block
                    # (splitting them onto ScalarE everywhere just delays exps
                    # behind copies on the strict-FIFO ACT queue — measured)
                    if i == n_qb - 1 and t % 2 == 1:
                        nc.scalar.copy(o_sb, o_ps)
                    else:
                        nc.vector.tensor_copy(o_sb, o_ps)
                    nc.sync.dma_start(
                        out=outT[t * 128 : (t + 1) * 128, s0 : s0 + CH],
                        in_=o_sb,
                    )

            # ---- main loop ----
            # The Tile scheduler is readiness-driven per engine: later-emitted
            # projection matmuls naturally fill exp-wait bubbles inside the
            # current block's attention, so plain program order works best.
            for i in range(n_qb):
                if i == 0:
                    cos_t, sin_t, xbig = cos0, sin0, xbig0
                else:
                    s0 = i * CH
                    cos_t = ropepool.tile([128, CH], f32, tag="cos", name=f"cos{i}")
                    nc.sync.dma_start(out=cos_t, in_=cosT[:, s0 : s0 + CH])
                    sin_t = ropepool.tile([128, CH], f32, tag="sin", name=f"sin{i}")
                    nc.sync.dma_start(out=sin_t, in_=sinTs[:, s0 : s0 + CH])
                    xbig = xpool.tile([128, DT, CH], bf16, tag="xt", name=f"xt{i}")
                    nc.sync.dma_start(out=xbig, in_=xTt[:, i])
                q_t = emit_projections(i, cos_t, sin_t, xbig)
                if i == 0:
                    # lower-priority loads, after the first chunk's stream
                    w_o_sb = wpool.tile([128, 2, D], bf16)
                    nc.sync.dma_start(out=w_o_sb, in_=w_oTt[:, :, :])
                    mask_sb = consts.tile([128, 128], bf16)
                    nc.sync.dma_start(out=mask_sb, in_=maskp[:, :])
                emit_attention(i, q_t)

    nc.finalize()
    return nc


def _host_inputs(x, w_q, w_k, w_v, w_o, s_len):
    """Host-side sharding / layout prep. Returns per-core input maps."""
    import ml_dtypes

    bf = ml_dtypes.bfloat16
    x2 = np.ascontiguousarray(x.reshape(s_len, D).astype(np.float32))
    xT = np.ascontiguousarray(x2.T.astype(bf))

    half = 64
    inv_freq = 1.0 / (10000.0 ** (np.arange(half, dtype=np.float32) / half))
    pos = np.arange(s_len, dtype=np.float32)
    ang = pos[:, None] * inv_freq[None, :]
    ang = np.concatenate([ang, ang], axis=1)  # [s, 128]
    cosT = np.ascontiguousarray(np.cos(ang).T.astype(np.float32))
    sinTs = np.ascontiguousarray(np.sin(ang).T.astype(np.float32))
    sinTs[:half] *= -1.0

    kk = np.arange(128)[:, None]
    qq = np.arange(128)[None, :]
    maskp = (kk <= qq).astype(bf)  # [128, 128] local causal triangle

    # tiled layouts: [128, ...] partition-major so device DMAs are long
    # contiguous runs (descriptor-count-bound otherwise)
    xTt = np.ascontiguousarray(
        xT.reshape(16, 128, s_len // 512, 512).transpose(1, 2, 0, 3)
    )  # [128, n_ch, 16, 512]

    def wtile(wslice_T):  # [2048, 256] -> [128, 16, 256]
        return np.ascontiguousarray(wslice_T.reshape(16, 128, MLOC).transpose(1, 0, 2))

    in_maps = []
    for c in range(NCORES):
        rows = slice(MLOC * c, MLOC * (c + 1))
        w_oc = w_o[:, rows].T.astype(bf)  # [256, 2048]
        in_maps.append(
            {
                "xTt": xTt,
                "w_qTt": wtile(w_q[rows].T.astype(bf)),
                "w_kTt": wtile(w_k[rows].T.astype(bf)),
                "w_vTt": wtile(w_v[rows].T.astype(bf)),
                "w_oTt": np.ascontiguousarray(
                    w_oc.reshape(2, 128, D).transpose(1, 0, 2)
                ),
                "cosT": cosT,
                "sinTs": sinTs,
                "maskp": maskp,
            }
        )
    return in_maps


_NC_CACHE = {}


def kernel(x, w_q, w_k, w_v, w_o):
    from concourse.bass_utils import run_bass_kernel_spmd

    s_len = x.shape[1]
    if s_len not in _NC_CACHE:
        _NC_CACHE[s_len] = _build(s_len)
    nc = _NC_CACHE[s_len]

    in_maps = _host_inputs(
        np.asarray(x), np.asarray(w_q), np.asarray(w_k), np.asarray(w_v),
        np.asarray(w_o), s_len,
    )
    res = run_bass_kernel_spmd(nc, in_maps, core_ids=list(range(NCORES)))
    acc = np.zeros((D, s_len), dtype=np.float32)
    for r in res.results:
        acc += r["outT"]
    return np.ascontiguousarray(acc.T)[None].astype(np.float32)



# revision 22
# speedup vs baseline: 1.0289x; 1.0240x over previous
"""Multi-head causal attention with RoPE on 8 Trainium2 NeuronCores.

Sharding: tensor-parallel over heads. Each of the 8 cores owns 2 of the 16
heads (a 256-row slice of w_q/w_k/w_v and the matching 256-column slice of
w_o). x is replicated. Each core computes its partial output projection
outT_c = w_o_slice.T @ ctx_slice in transposed [d, s] layout; the host sums
the 8 partials and transposes back.

On-device layout strategy (per core):
  - x arrives transposed: xT [2048, 4096] so projections contract d on
    partitions. q/k are produced in [head_dim, s] layout (RoPE applied via
    partition-half swizzle on DVE); v in natural [s, head_dim] layout.
  - attention runs in "scoresT" geometry: scoresT[k, q] = kT_tile.T @ qT,
    exp on ScalarE over k-tile PAIRS (scale 1/sqrt(128) fused, no max
    subtraction - scaled scores are bounded ~|6.5| so fp32 exp is safe),
    PV accumulates ctxT[m, q] with v tiles stationary, softmax denominator
    via an all-ones [128,128] stationary matmul (gives the denominator
    pre-broadcast across partitions), then reciprocal_approx_fast and one
    multiply normalize the context.
  - matmul operands are bf16 (full PE rate); all accumulation is fp32 PSUM.
Causality: per 512-wide q block only the valid k tiles run; scores/exp/PV/
denominator on diagonal tiles run only over the valid column suffix, and the
single partial 128-col triangle is masked post-exp with a [128,128] mask.
"""

import math

import numpy as np

S = 4096
D = 2048
DT = D // 128           # 16 d-tiles
MLOC = 256              # head dims per core (2 heads x 128)
HLOC = 2                # heads per core
CH = 512                # s-chunk == attention q-block
NCORES = 8
SCALE = 1.0 / math.sqrt(128.0)


def _build(s_len):
    import concourse.mybir as mybir
    import concourse.tile as tile
    from concourse import bacc

    f32 = mybir.dt.float32
    bf16 = mybir.dt.bfloat16
    AF = mybir.ActivationFunctionType

    n_qb = s_len // CH
    n_st = s_len // 128

    # partition id is unused (inputs are host-sharded per core); disabling it
    # removes the per-engine TENSOR_LOADs from the kernel preamble
    nc = bacc.Bacc(enable_partition_id=False)

    xTt = nc.dram_tensor("xTt", [128, s_len // CH, DT, CH], bf16, kind="ExternalInput")
    w_qTt = nc.dram_tensor("w_qTt", [128, DT, MLOC], bf16, kind="ExternalInput")
    w_kTt = nc.dram_tensor("w_kTt", [128, DT, MLOC], bf16, kind="ExternalInput")
    w_vTt = nc.dram_tensor("w_vTt", [128, DT, MLOC], bf16, kind="ExternalInput")
    w_oTt = nc.dram_tensor("w_oTt", [128, 2, D], bf16, kind="ExternalInput")
    cosT = nc.dram_tensor("cosT", [128, s_len], f32, kind="ExternalInput")
    sinTs = nc.dram_tensor("sinTs", [128, s_len], f32, kind="ExternalInput")
    # local causal triangle for diagonal k-tiles: tri[k, q'] = 1 iff k <= q'.
    # With suffix-sliced exp, only the first 128 columns past each diagonal
    # tile's q-offset ever need masking, and that triangle is the same for
    # every diagonal tile.
    maskp = nc.dram_tensor("maskp", [128, 128], bf16, kind="ExternalInput")
    outT = nc.dram_tensor("outT", [D, s_len], f32, kind="ExternalOutput")

    from contextlib import ExitStack

    with tile.TileContext(nc) as tc:
        with ExitStack() as ctx:
            consts = ctx.enter_context(tc.tile_pool(name="consts", bufs=1))
            wpool = ctx.enter_context(tc.tile_pool(name="wpool", bufs=1))
            kvpool = ctx.enter_context(tc.tile_pool(name="kvpool", bufs=1))
            xpool = ctx.enter_context(tc.tile_pool(name="xpool", bufs=2))
            qpool = ctx.enter_context(tc.tile_pool(name="qpool", bufs=2))
            ropepool = ctx.enter_context(tc.tile_pool(name="ropepool", bufs=2))
            tmppool = ctx.enter_context(tc.tile_pool(name="tmppool", bufs=3))
            epool = ctx.enter_context(tc.tile_pool(name="epool", bufs=6))
            spool = ctx.enter_context(tc.tile_pool(name="spool", bufs=4))
            ctxnpool = ctx.enter_context(tc.tile_pool(name="ctxnpool", bufs=2))
            rfpool = ctx.enter_context(tc.tile_pool(name="rfpool", bufs=2))
            obuf = ctx.enter_context(tc.tile_pool(name="obuf", bufs=3))
            pp1 = ctx.enter_context(tc.tile_pool(name="pp1", bufs=2, space="PSUM"))
            psc = ctx.enter_context(tc.tile_pool(name="psc", bufs=2, space="PSUM"))
            pam = ctx.enter_context(tc.tile_pool(name="pam", bufs=2, space="PSUM"))

            # ---- weights needed immediately ----
            # fine-grained first transfers: the first q-projection matmul only
            # needs w_q[:, 0:2, :] and x[:, 0, 0:2, :], so land those first and
            # stream the rest behind them to cut the startup PE bubble.
            # Startup transfers split across the two HWDGE issue engines (Sync
            # and Scalar) so descriptor generation and the two HW queues run
            # in parallel: Sync streams w_q + x chunk 0 (first matmul deps),
            # Scalar streams cos/sin (needed by rope at ~14us), then w_k, w_v.
            w_q_sb = wpool.tile([128, DT, MLOC], bf16)
            nc.sync.dma_start(out=w_q_sb[:, 0:2], in_=w_qTt[:, 0:2, :])
            xbig0 = xpool.tile([128, DT, CH], bf16, tag="xt", name="xt_pre0")
            nc.sync.dma_start(out=xbig0[:, 0:2], in_=xTt[:, 0, 0:2])
            cos0 = ropepool.tile([128, CH], f32, tag="cos", name="cos_pre0")
            nc.scalar.dma_start(out=cos0, in_=cosT[:, 0:CH])
            sin0 = ropepool.tile([128, CH], f32, tag="sin", name="sin_pre0")
            nc.scalar.dma_start(out=sin0, in_=sinTs[:, 0:CH])
            nc.sync.dma_start(out=w_q_sb[:, 2:8], in_=w_qTt[:, 2:8, :])
            nc.sync.dma_start(out=xbig0[:, 2:8], in_=xTt[:, 0, 2:8])
            w_k_sb = wpool.tile([128, DT, MLOC], bf16)
            nc.scalar.dma_start(out=w_k_sb, in_=w_kTt[:, :, :])
            nc.sync.dma_start(out=w_q_sb[:, 8:DT], in_=w_qTt[:, 8:DT, :])
            nc.sync.dma_start(out=xbig0[:, 8:DT], in_=xTt[:, 0, 8:DT])
            w_v_sb = wpool.tile([128, DT, MLOC], bf16)
            nc.scalar.dma_start(out=w_v_sb, in_=w_vTt[:, :, :])
            ones_sb = consts.tile([128, 128], bf16)
            nc.vector.memset(ones_sb, 1.0)

            # HAM pre-warm: the PE clock gate opens only after ~3.4us of
            # sustained matmul activity. Burn that window on dummy matmuls
            # during the initial DMA wait so the real projections start at
            # full clock. Output goes to a scratch PSUM tile, never read.
            warm_sb = consts.tile([128, CH], bf16)
            nc.vector.memset(warm_sb, 0.0)
            warm_ps = pam.tile([128, CH], f32, tag="am", name="warm_ps")
            for w in range(9):
                nc.tensor.matmul(
                    warm_ps,
                    lhsT=ones_sb,
                    rhs=warm_sb,
                    start=True,
                    stop=True,
                    skip_group_check=True,
                )

            # ---- persistent per-core tensors ----
            kT_sb = []
            for h in range(HLOC):
                kt = kvpool.tile([128, s_len], bf16, name=f"kT{h}")
                kT_sb.append(kt)
            v_sb = kvpool.tile([128, n_st * MLOC], bf16)

            w_o_sb = None
            mask_sb = None

            def emit_projections(i, cos_t, sin_t, xbig):
                """QKV projections + rope for s-chunk i. Returns per-head q tiles."""
                s0 = i * CH
                q_t = []
                for h in range(HLOC):
                    for wname, w_sb in (("q", w_q_sb), ("k", w_k_sb)):
                        acc = pp1.tile(
                            [128, CH], f32, tag="pp1", name=f"acc_{wname}{h}_{i}"
                        )
                        for d in range(DT):
                            nc.tensor.matmul(
                                acc,
                                lhsT=w_sb[:, d, h * 128 : (h + 1) * 128],
                                rhs=xbig[:, d, :],
                                start=(d == 0),
                                stop=(d == DT - 1),
                            )
                        if wname == "q":
                            dst = qpool.tile(
                                [128, CH], bf16, tag=f"q{h}", name=f"q{h}_{i}"
                            )
                            q_t.append(dst)
                        else:
                            dst = kT_sb[h][:, s0 : s0 + CH]
                        # rope: dst = acc*cos + rot(acc)*sin_signed
                        t1 = tmppool.tile(
                            [128, CH], f32, tag="t1", name=f"t1_{wname}{h}_{i}"
                        )
                        nc.vector.tensor_mul(t1, acc, cos_t)
                        nc.vector.tensor_mul(dst[0:64], acc[64:128], sin_t[0:64])
                        nc.vector.tensor_mul(dst[64:128], acc[0:64], sin_t[64:128])
                        nc.vector.tensor_add(dst, dst, t1)

                for st in range(CH // 128):
                    vacc = pp1.tile([128, MLOC], f32, tag="pp1", name=f"vacc{st}_{i}")
                    for d in range(DT):
                        nc.tensor.matmul(
                            vacc,
                            lhsT=xbig[:, d, st * 128 : (st + 1) * 128],
                            rhs=w_v_sb[:, d, :],
                            start=(d == 0),
                            stop=(d == DT - 1),
                        )
                    gst = i * (CH // 128) + st
                    nc.vector.tensor_copy(v_sb[:, gst * MLOC : (gst + 1) * MLOC], vacc)
                return q_t

            def emit_attention(i, q_t):
                """Attention + output projection for q-block i."""
                s0 = i * CH
                nk = (i + 1) * (CH // 128)
                npair = nk // 2
                ctxn = []
                for h in range(HLOC):
                    ctx_ps = pam.tile([128, CH], f32, tag="am", name=f"ctx{h}_{i}")
                    den_ps = pam.tile([128, CH], f32, tag="am", name=f"den{h}_{i}")
                    for jp in range(npair):
                        p0 = 2 * jp - (CH // 128) * i  # diagonal pattern of half 0
                        qlo0 = max(p0, 0) * 128
                        sc = psc.tile(
                            [128, 2, CH], f32, tag="sc", name=f"sc{h}_{i}_{jp}"
                        )
                        for half in range(2):
                            j = 2 * jp + half
                            p = j - (CH // 128) * i
                            qlo = max(p, 0) * 128
                            nc.tensor.matmul(
                                sc[:, half, qlo:CH],
                                lhsT=kT_sb[h][:, j * 128 : (j + 1) * 128],
                                rhs=q_t[h][:, qlo:CH],
                                start=True,
                                stop=True,
                            )
                        # exp per pair on ScalarE. The first pair of a block is
                        # split per half to cut the scores->exp->PV pipeline
                        # fill latency; the rest run as one flat ACT (per-ACT
                        # overhead beats the column savings). On diagonal pairs
                        # the flat ACT eats stale PSUM in the pre-suffix
                        # columns, which the masks/memset below neutralize
                        # before anything reads them.
                        e = epool.tile([128, 2, CH], bf16, tag="e", name=f"e{h}_{i}_{jp}")
                        nc.scalar.activation(
                            e.rearrange("p a b -> p (a b)"),
                            sc.rearrange("p a b -> p (a b)"),
                            AF.Exp,
                            scale=SCALE,
                        )
                        if p0 >= 0:
                            # half 0: triangle at its diagonal tile
                            nc.vector.tensor_mul(
                                e[:, 0, qlo0 : qlo0 + 128],
                                e[:, 0, qlo0 : qlo0 + 128],
                                mask_sb,
                            )
                            # half 1: zero the 128 cols before its diagonal tile
                            # (so the pair-sum below stays causal), then triangle
                            nc.vector.memset(e[:, 1, qlo0 : qlo0 + 128], 0.0)
                            nc.vector.tensor_mul(
                                e[:, 1, qlo0 + 128 : qlo0 + 256],
                                e[:, 1, qlo0 + 128 : qlo0 + 256],
                                mask_sb,
                            )
                        # pair-sum for the denominator: one den matmul per pair
                        # (halves the ones-matmul count on the PE; the sum runs
                        # on the DVE, whose latency the scheduler hides better
                        # than GpSimd's)
                        s_p = spool.tile(
                            [128, CH], bf16, tag="sp", name=f"sp{h}_{i}_{jp}"
                        )
                        nc.vector.tensor_add(
                            s_p[:, qlo0:CH], e[:, 0, qlo0:CH], e[:, 1, qlo0:CH]
                        )
                        nc.tensor.matmul(
                            den_ps[:, qlo0:CH],
                            lhsT=ones_sb,
                            rhs=s_p[:, qlo0:CH],
                            start=(jp == 0),
                            stop=(jp == npair - 1),
                            skip_group_check=True,
                        )
                        for half in range(2):
                            j = 2 * jp + half
                            p = j - (CH // 128) * i
                            qlo = max(p, 0) * 128  # valid column suffix start
                            nc.tensor.matmul(
                                ctx_ps[:, qlo:CH],
                                lhsT=v_sb[
                                    :, j * MLOC + h * 128 : j * MLOC + (h + 1) * 128
                                ],
                                rhs=e[:, half, qlo:CH],
                                start=(j == 0),
                                stop=(j == nk - 1),
                                skip_group_check=True,
                            )
                    rf = rfpool.tile([128, CH], f32, tag="rf", name=f"rf{h}_{i}")
                    nc.vector.reciprocal_approx_fast(rf, den_ps)
                    cn = ctxnpool.tile([128, CH], bf16, tag=f"cn{h}", name=f"cn{h}_{i}")
                    nc.vector.tensor_mul(cn, ctx_ps, rf)
                    ctxn.append(cn)
                return ctxn

            def emit_outproj(i, ctxn, opool, otag, scalar_split):
                s0 = i * CH
                for t in range(DT):
                    o_ps = opool.tile([128, CH], f32, tag=otag, name=f"o{t}_{i}")
                    for ot in range(2):
                        nc.tensor.matmul(
                            o_ps,
                            lhsT=w_o_sb[:, ot, t * 128 : (t + 1) * 128],
                            rhs=ctxn[ot],
                            start=(ot == 0),
                            stop=(ot == 1),
                        )
                    o_sb = obuf.tile([128, CH], f32, tag="osb", name=f"osb{t}_{i}")
                    # PSUM->SBUF drains stay on DVE except in the last block
                    # (splitting them onto ScalarE everywhere just delays exps
                    # behind copies on the strict-FIFO ACT queue — measured)
                    if scalar_split and t % 2 == 1:
                        nc.scalar.copy(o_sb, o_ps)
                    else:
                        nc.vector.tensor_copy(o_sb, o_ps)
                    nc.sync.dma_start(
                        out=outT[t * 128 : (t + 1) * 128, s0 : s0 + CH],
                        in_=o_sb,
                    )

            # ---- main loop ----
            # The Tile scheduler is readiness-driven per engine: later-emitted
            # projection matmuls naturally fill exp-wait bubbles inside the
            # current block's attention, so plain program order works best.
            # Exception: the LAST block's attention is exp-bound (32 ACTs =
            # 35.6us vs ~24us of PE work) with no later projections to fill
            # the gap, so block n_qb-2's output projection is deferred past
            # its pairs — those 32 ready matmuls become the filler. Its PSUM
            # comes from the projection pool, which is idle by then.
            ctxn_defer = None
            for i in range(n_qb):
                if i == 0:
                    cos_t, sin_t, xbig = cos0, sin0, xbig0
                else:
                    s0 = i * CH
                    cos_t = ropepool.tile([128, CH], f32, tag="cos", name=f"cos{i}")
                    nc.sync.dma_start(out=cos_t, in_=cosT[:, s0 : s0 + CH])
                    sin_t = ropepool.tile([128, CH], f32, tag="sin", name=f"sin{i}")
                    nc.sync.dma_start(out=sin_t, in_=sinTs[:, s0 : s0 + CH])
                    xbig = xpool.tile([128, DT, CH], bf16, tag="xt", name=f"xt{i}")
                    nc.sync.dma_start(out=xbig, in_=xTt[:, i])
                q_t = emit_projections(i, cos_t, sin_t, xbig)
                if i == 0:
                    # lower-priority loads, after the first chunk's stream
                    w_o_sb = wpool.tile([128, 2, D], bf16)
                    nc.sync.dma_start(out=w_o_sb, in_=w_oTt[:, :, :])
                    mask_sb = consts.tile([128, 128], bf16)
                    nc.sync.dma_start(out=mask_sb, in_=maskp[:, :])
                ctxn = emit_attention(i, q_t)
                if n_qb >= 2 and i == n_qb - 2:
                    ctxn_defer = ctxn
                elif i == n_qb - 1:
                    if ctxn_defer is not None:
                        emit_outproj(n_qb - 2, ctxn_defer, pp1, "pp1", False)
                    emit_outproj(i, ctxn, pam, "am", True)
                else:
                    emit_outproj(i, ctxn, pam, "am", False)

    nc.finalize()
    return nc


def _host_inputs(x, w_q, w_k, w_v, w_o, s_len):
    """Host-side sharding / layout prep. Returns per-core input maps."""
    import ml_dtypes

    bf = ml_dtypes.bfloat16
    x2 = np.ascontiguousarray(x.reshape(s_len, D).astype(np.float32))
    xT = np.ascontiguousarray(x2.T.astype(bf))

    half = 64
    inv_freq = 1.0 / (10000.0 ** (np.arange(half, dtype=np.float32) / half))
    pos = np.arange(s_len, dtype=np.float32)
    ang = pos[:, None] * inv_freq[None, :]
    ang = np.concatenate([ang, ang], axis=1)  # [s, 128]
    cosT = np.ascontiguousarray(np.cos(ang).T.astype(np.float32))
    sinTs = np.ascontiguousarray(np.sin(ang).T.astype(np.float32))
    sinTs[:half] *= -1.0

    kk = np.arange(128)[:, None]
    qq = np.arange(128)[None, :]
    maskp = (kk <= qq).astype(bf)  # [128, 128] local causal triangle

    # tiled layouts: [128, ...] partition-major so device DMAs are long
    # contiguous runs (descriptor-count-bound otherwise)
    xTt = np.ascontiguousarray(
        xT.reshape(16, 128, s_len // 512, 512).transpose(1, 2, 0, 3)
    )  # [128, n_ch, 16, 512]

    def wtile(wslice_T):  # [2048, 256] -> [128, 16, 256]
        return np.ascontiguousarray(wslice_T.reshape(16, 128, MLOC).transpose(1, 0, 2))

    in_maps = []
    for c in range(NCORES):
        rows = slice(MLOC * c, MLOC * (c + 1))
        w_oc = w_o[:, rows].T.astype(bf)  # [256, 2048]
        in_maps.append(
            {
                "xTt": xTt,
                "w_qTt": wtile(w_q[rows].T.astype(bf)),
                "w_kTt": wtile(w_k[rows].T.astype(bf)),
                "w_vTt": wtile(w_v[rows].T.astype(bf)),
                "w_oTt": np.ascontiguousarray(
                    w_oc.reshape(2, 128, D).transpose(1, 0, 2)
                ),
                "cosT": cosT,
                "sinTs": sinTs,
                "maskp": maskp,
            }
        )
    return in_maps


_NC_CACHE = {}


def kernel(x, w_q, w_k, w_v, w_o):
    from concourse.bass_utils import run_bass_kernel_spmd

    s_len = x.shape[1]
    if s_len not in _NC_CACHE:
        _NC_CACHE[s_len] = _build(s_len)
    nc = _NC_CACHE[s_len]

    in_maps = _host_inputs(
        np.asarray(x), np.asarray(w_q), np.asarray(w_k), np.asarray(w_v),
        np.asarray(w_o), s_len,
    )
    res = run_bass_kernel_spmd(nc, in_maps, core_ids=list(range(NCORES)))
    acc = np.zeros((D, s_len), dtype=np.float32)
    for r in res.results:
        acc += r["outT"]
    return np.ascontiguousarray(acc.T)[None].astype(np.float32)

